# revision 1
# baseline (speedup 1.0000x reference)
"""Trainium2 Bass kernel for nn_ADConv (adaptive-basis conv).

Math (per image, per pixel q=(h,w)):
  h1  = tanh(bn1(conv3x3(x)))                      # [64, H, W]
  bc  = tanh(bn2(conv3x3(h1)))                     # [96, H, W], channel = 6f+t
  PB[c,t,q]   = sum_k x[c, q+dk] * B[t,k]          # depthwise basis conv
  u[c,f,q]    = sum_t PB[c,t,q] * bc[6f+t, wq, hq] # per-pixel bilinear (DVE)
  out[o,w,h]  = sum_{c,f} coef[o, 16c+f] * u[c,f,q]

Sharding: data-parallel, batch 16 -> 2 images per NeuronCore, params
replicated. Everything computed in bf16 (fp32 PSUM accumulation).
"""

import os
import sys

import numpy as np

sys.path.insert(0, "/opt/trn_rl_repo")

import ml_dtypes

import concourse.bacc as bacc
import concourse.bass as bass
import concourse.mybir as mybir
import concourse.tile as tile
from concourse.ap import AP
from concourse.bass_utils import run_bass_kernel_spmd

BF16 = mybir.dt.bfloat16
F32 = mybir.dt.float32
AF = mybir.ActivationFunctionType
ALU = mybir.AluOpType

N_CORES = 8
IMGS = 2           # images per core
C = 64             # input channels
INTER = 64         # conv1 out channels
BCH = 96           # conv2 out channels = 16f * 6t
NT = 6             # TOTAL_BASES
NF = 16            # NUM_FA
O = 128            # output channels
H = W = 64
HP = 66            # padded spatial
Q = H * W          # 4096 pixels
RC = 8             # rows per chunk
NCHUNK = H // RC   # 8 chunks of 512 px
CH = RC * W        # 512 px per chunk
BN_EPS = 1e-5

_CACHE = {}


def _pbcast_src(tile_ap: AP, part_row: int, part_pitch: int, dims, offset_elems: int):
    """Manual AP: read from partition `part_row` of an SBUF tile, broadcast
    across 64 partitions (leading 0-stride dim), with free dims `dims`
    (list of [step, count]) starting at byte/elem offset `offset_elems`."""
    base = tile_ap  # AP covering whole tile, standard layout
    # Flat-element convention: partition p lives at p * part_pitch.
    off = part_row * part_pitch + offset_elems
    return AP(base.tensor, base.offset + off, [[0, 64]] + list(dims))


def build_graph():
    nc = bacc.Bacc(None, target_bir_lowering=False)

    xs = nc.declare_dram_parameter("xs", [IMGS, C, H, W], BF16, isOutput=False)
    w1p = nc.declare_dram_parameter("w1p", [128, 3, INTER], BF16, isOutput=False)
    w1s = nc.declare_dram_parameter("w1s", [C, 3, INTER], BF16, isOutput=False)
    s1 = nc.declare_dram_parameter("s1", [INTER, 1], F32, isOutput=False)
    b1 = nc.declare_dram_parameter("b1", [INTER, 1], F32, isOutput=False)
    w2p = nc.declare_dram_parameter("w2p", [128, 3, BCH], BF16, isOutput=False)
    w2s = nc.declare_dram_parameter("w2s", [INTER, 3, BCH], BF16, isOutput=False)
    s2 = nc.declare_dram_parameter("s2", [BCH, 1], F32, isOutput=False)
    b2 = nc.declare_dram_parameter("b2", [BCH, 1], F32, isOutput=False)
    wpbp = nc.declare_dram_parameter("wpbp", [128, 3, NT * C], BF16, isOutput=False)
    wpbs = nc.declare_dram_parameter("wpbs", [C, 3, NT * C], BF16, isOutput=False)
    coefp = nc.declare_dram_parameter("coefp", [128, NF, O], BF16, isOutput=False)
    out = nc.declare_dram_parameter("out", [IMGS, O, W, H], F32, isOutput=True)

    with tile.TileContext(nc) as tc:
        with (
            tc.tile_pool(name="persist", bufs=1) as pp,
            tc.tile_pool(name="bcrep", bufs=3) as bp,
            tc.tile_pool(name="uu", bufs=2) as up,
            tc.tile_pool(name="tmp", bufs=2) as tp,
            tc.tile_pool(name="osb", bufs=2) as op_,
            tc.tile_pool(name="dramb", bufs=1, space=bass.MemorySpace.DRAM) as dp,
            tc.tile_pool(name="ps_conv", bufs=1, space=bass.MemorySpace.PSUM) as pcv,
            tc.tile_pool(name="ps_pb", bufs=1, space=bass.MemorySpace.PSUM) as ppb,
            tc.tile_pool(name="ps_fin", bufs=1, space=bass.MemorySpace.PSUM) as pfn,
        ):
            # ---- persistent SBUF ----
            w1psb = pp.tile([128, 3, INTER], BF16, tag="w1psb")
            w1ssb = pp.tile([C, 3, INTER], BF16, tag="w1ssb")
            w2psb = pp.tile([128, 3, BCH], BF16, tag="w2psb")
            w2ssb = pp.tile([INTER, 3, BCH], BF16, tag="w2ssb")
            wpbpsb = pp.tile([128, 3, NT * C], BF16, tag="wpbpsb")
            wpbssb = pp.tile([C, 3, NT * C], BF16, tag="wpbssb")
            coefsb = pp.tile([128, NF, O], BF16, tag="coefsb")
            s1sb = pp.tile([INTER, 1], F32, tag="s1sb")
            b1sb = pp.tile([INTER, 1], F32, tag="b1sb")
            s2sb = pp.tile([BCH, 1], F32, tag="s2sb")
            b2sb = pp.tile([BCH, 1], F32, tag="b2sb")
            nc.sync.dma_start(w1psb[:], w1p[:])
            nc.sync.dma_start(w1ssb[:], w1s[:])
            nc.sync.dma_start(w2psb[:], w2p[:])
            nc.sync.dma_start(w2ssb[:], w2s[:])
            nc.sync.dma_start(wpbpsb[:], wpbp[:])
            nc.sync.dma_start(wpbssb[:], wpbs[:])
            nc.sync.dma_start(coefsb[:], coefp[:])
            nc.sync.dma_start(s1sb[:], s1[:])
            nc.sync.dma_start(b1sb[:], b1[:])
            nc.sync.dma_start(s2sb[:], s2[:])
            nc.sync.dma_start(b2sb[:], b2[:])

            xpad = []
            hpad = []
            bcs = []
            for i in range(IMGS):
                xp = pp.tile([128, HP, HP], BF16, tag=f"xpad{i}", name=f"xpad{i}")
                hp = pp.tile([128, HP, HP], BF16, tag=f"hpad{i}", name=f"hpad{i}")
                bi = pp.tile([BCH, H, W], BF16, tag=f"bc{i}", name=f"bc{i}")
                nc.vector.memset(xp[:], 0.0)
                nc.vector.memset(hp[:], 0.0)
                nc.sync.dma_start(xp[0:64, 1 : H + 1, 1 : W + 1], xs[i])
                nc.sync.dma_start(xp[64:128, 1 : H + 1, 0:W], xs[i])
                xpad.append(xp)
                hpad.append(hp)
                bcs.append(bi)
            pbt2 = [
                [
                    pp.tile([128, Q], BF16, tag=f"pbt{i}_{j3}", name=f"pbt{i}_{j3}")
                    for j3 in range(3)
                ]
                for i in range(IMGS)
            ]
            bcd = [
                dp.tile([NCHUNK, BCH, CH], BF16, tag=f"bcd{i}", name=f"bcd{i}")
                for i in range(IMGS)
            ]

            # ---- stage 1+2+PB per image ----
            for i in range(IMGS):
                # conv1 -> bn -> tanh -> hpad
                for g in range(4):  # chunk groups of 2 (PSUM budget)
                    cps = pcv.tile([INTER, 2, RC, W], F32, tag="convps")
                    for m in range(6):
                        ki = m % 3
                        pair = m < 3
                        lhsT = w1psb[:, ki, :] if pair else w1ssb[:, ki, :]
                        for j in range(2):
                            h0 = (g * 2 + j) * RC
                            if pair:  # taps (ki,0)+(ki,1): hi half pre-shifted
                                rhs = xpad[i][:, h0 + ki : h0 + ki + RC, 0:W]
                            else:     # tap (ki,2)
                                rhs = xpad[i][0:64, h0 + ki : h0 + ki + RC, 2 : 2 + W]
                            nc.tensor.matmul(
                                cps[:, j], lhsT, rhs, start=(m == 0), stop=(m == 5)
                            )
                    for j in range(2):
                        h0 = (g * 2 + j) * RC
                        nc.scalar.activation(
                            hpad[i][0:64, h0 + 1 : h0 + 1 + RC, 1 : W + 1],
                            cps[:, j],
                            AF.Tanh,
                            bias=b1sb[:],
                            scale=s1sb[:],
                        )
                        nc.scalar.activation(
                            hpad[i][64:128, h0 + 1 : h0 + 1 + RC, 0:W],
                            cps[:, j],
                            AF.Tanh,
                            bias=b1sb[:],
                            scale=s1sb[:],
                        )
                # conv2 -> bn -> tanh -> bc
                for g in range(4):
                    cps = pcv.tile([BCH, 2, RC, W], F32, tag="convps")
                    for m in range(6):
                        ki = m % 3
                        pair = m < 3
                        lhsT = w2psb[:, ki, :] if pair else w2ssb[:, ki, :]
                        for j in range(2):
                            h0 = (g * 2 + j) * RC
                            if pair:
                                rhs = hpad[i][:, h0 + ki : h0 + ki + RC, 0:W]
                            else:
                                rhs = hpad[i][0:64, h0 + ki : h0 + ki + RC, 2 : 2 + W]
                            nc.tensor.matmul(
                                cps[:, j], lhsT, rhs, start=(m == 0), stop=(m == 5)
                            )
                    for j in range(2):
                        h0 = (g * 2 + j) * RC
                        # transposed store: bcT[ch, w_out, h_bc] so later
                        # per-pixel reads are contiguous
                        nc.scalar.activation(
                            bcs[i][:, :, h0 : h0 + RC].transpose([0, 2, 1]),
                            cps[:, j],
                            AF.Tanh,
                            bias=b2sb[:],
                            scale=s2sb[:],
                        )
            for i in range(IMGS):
                for ch in range(NCHUNK):
                    nc.sync.dma_start(
                        bcd[i][ch], bcs[i][:, ch * RC : (ch + 1) * RC, :].opt()
                    )

            # ---- per chunk: PB (PE) -> products (DVE) -> final w/ t-fold (PE) ----
            UCH = 2 * CH  # 1024-px u-chunks
            for uc in range(NCHUNK // 2):
                for half in range(2):
                    ch = uc * 2 + half
                    h0 = ch * RC
                    c0 = h0 * W
                    for i in range(IMGS):
                        for j3 in range(3):
                            pps = ppb.tile(
                                [128, RC, W], F32, tag="pbps", name="pps", bufs=2
                            )
                            for m in range(6):
                                ki = m % 3
                                pair = m < 3
                                if pair:
                                    lhsT = wpbpsb[:, ki, j3 * 128 : (j3 + 1) * 128]
                                    rhs = xpad[i][:, h0 + ki : h0 + ki + RC, 0:W]
                                else:
                                    lhsT = wpbssb[:, ki, j3 * 128 : (j3 + 1) * 128]
                                    rhs = xpad[i][0:64, h0 + ki : h0 + ki + RC, 2 : 2 + W]
                                nc.tensor.matmul(
                                    pps[:], lhsT, rhs, start=(m == 0), stop=(m == 5)
                                )
                            nc.vector.tensor_copy(
                                pbt2[i][j3][:, c0 : c0 + CH], pps[:].opt()
                            )
                u0 = uc * UCH
                fps = [
                    pfn.tile([O, 2, CH], F32, tag=f"finps{i}", name=f"fps{i}", bufs=1)
                    for i in range(IMGS)
                ]
                for f in range(NF):
                    for i in range(IMGS):
                        # bcr_pair[p, j3, half, q]: p<64 <- bc row (f*6+2*j3),
                        # p>=64 <- bc row (f*6+2*j3+1), replicated across 64
                        bcr = bp.tile(
                            [128, 3 * UCH], BF16, tag="bcrv", name="bcrv", bufs=4
                        )
                        bcr_full = bcr[:]
                        dfull = bcd[i][:]
                        for ph in range(2):
                            for half in range(2):
                                srcap = AP(
                                    dfull.tensor,
                                    dfull.offset
                                    + (uc * 2 + half) * BCH * CH
                                    + (f * NT + ph) * CH,
                                    [[0, 64], [2 * CH, 3], [1, CH]],
                                )
                                dstap = AP(
                                    bcr_full.tensor,
                                    bcr_full.offset
                                    + ph * 64 * (3 * UCH)
                                    + half * CH,
                                    [[3 * UCH, 64], [UCH, 3], [1, CH]],
                                )
                                iss = (nc.sync, nc.scalar, nc.gpsimd)[
                                    (f + 2 * i + 2 * ph + half) % 3
                                ]
                                iss.dma_start(dstap, srcap)
                        theta = tp.tile(
                            [128, 3 * UCH], BF16, tag="theta", name="theta", bufs=4
                        )
                        for j3 in range(3):
                            nc.vector.tensor_mul(
                                theta[:, j3 * UCH : (j3 + 1) * UCH],
                                pbt2[i][j3][:, u0 : u0 + UCH],
                                bcr[:, j3 * UCH : (j3 + 1) * UCH],
                            )
                        for j3 in range(3):
                            for half in range(2):
                                nc.tensor.matmul(
                                    fps[i][:, half],
                                    coefsb[:, f, :],
                                    theta[
                                        :,
                                        j3 * UCH + half * CH : j3 * UCH + (half + 1) * CH,
                                    ],
                                    start=(f == 0 and j3 == 0),
                                    stop=(f == NF - 1 and j3 == 2),
                                )
                for i in range(IMGS):
                    for half in range(2):
                        ch = uc * 2 + half
                        h0 = ch * RC
                        osb = op_.tile([O, W, RC], F32, tag="osb", name="osb")
                        nc.scalar.copy(
                            osb[:].transpose([0, 2, 1]),
                            fps[i][:, half].opt(),
                        )
                        nc.sync.dma_start(out[i, :, :, h0 : h0 + RC], osb[:])

    nc.compile()
    return nc


def _prep_params(inputs):
    bf16 = ml_dtypes.bfloat16
    f32 = np.float32
    c1w = np.asarray(inputs["conv1_w"], f32)
    c2w = np.asarray(inputs["conv2_w"], f32)
    bases = np.asarray(inputs["bases"], f32)
    coef = np.asarray(inputs["coef"], f32)

    s1 = np.asarray(inputs["bn1_gamma"], f32) / np.sqrt(
        np.asarray(inputs["bn1_var"], f32) + BN_EPS
    )
    b1 = (np.asarray(inputs["conv1_b"], f32) - np.asarray(inputs["bn1_mean"], f32)) * s1 + np.asarray(
        inputs["bn1_beta"], f32
    )
    s2 = np.asarray(inputs["bn2_gamma"], f32) / np.sqrt(
        np.asarray(inputs["bn2_var"], f32) + BN_EPS
    )
    b2 = (np.asarray(inputs["conv2_b"], f32) - np.asarray(inputs["bn2_mean"], f32)) * s2 + np.asarray(
        inputs["bn2_beta"], f32
    )

    w1pk = np.zeros((128, 3, INTER), f32)
    w1sk = np.zeros((C, 3, INTER), f32)
    w2pk = np.zeros((128, 3, BCH), f32)
    w2sk = np.zeros((INTER, 3, BCH), f32)
    for ki in range(3):
        w1pk[0:64, ki] = c1w[:, :, ki, 0].T
        w1pk[64:128, ki] = c1w[:, :, ki, 1].T
        w1sk[:, ki] = c1w[:, :, ki, 2].T
        w2pk[0:64, ki] = c2w[:, :, ki, 0].T
        w2pk[64:128, ki] = c2w[:, :, ki, 1].T
        w2sk[:, ki] = c2w[:, :, ki, 2].T

    wpbpk = np.zeros((128, 3, NT * C), f32)
    wpbsk = np.zeros((C, 3, NT * C), f32)
    for t in range(NT):
        for c in range(C):
            for ki in range(3):
                wpbpk[c, ki, t * C + c] = bases[t, 3 * ki + 0]
                wpbpk[64 + c, ki, t * C + c] = bases[t, 3 * ki + 1]
                wpbsk[c, ki, t * C + c] = bases[t, 3 * ki + 2]

    cview = coef.reshape(O, C, NF)  # coef[o, 16c+f]
    coefp64 = np.ascontiguousarray(cview.transpose(1, 2, 0))  # [c, f, o]
    coefp = np.concatenate([coefp64, coefp64], axis=0)  # [128, f, o]

    return {
        "w1p": w1pk.astype(bf16),
        "w1s": w1sk.astype(bf16),
        "s1": s1.reshape(-1, 1).astype(f32),
        "b1": b1.reshape(-1, 1).astype(f32),
        "w2p": w2pk.astype(bf16),
        "w2s": w2sk.astype(bf16),
        "s2": s2.reshape(-1, 1).astype(f32),
        "b2": b2.reshape(-1, 1).astype(f32),
        "wpbp": wpbpk.astype(bf16),
        "wpbs": wpbsk.astype(bf16),
        "coefp": coefp.astype(bf16),
    }


def kernel(**inputs):
    if "nc" not in _CACHE:
        _CACHE["nc"] = build_graph()
    nc = _CACHE["nc"]

    params = _prep_params(inputs)
    x = np.asarray(inputs["x"], np.float32).astype(ml_dtypes.bfloat16)

    in_maps = []
    for core in range(N_CORES):
        m = dict(params)
        m["xs"] = np.ascontiguousarray(x[core * IMGS : (core + 1) * IMGS])
        in_maps.append(m)

    res = run_bass_kernel_spmd(nc, in_maps, core_ids=list(range(N_CORES)))
    outs = [r["out"] for r in res.results]
    return np.concatenate(outs, axis=0).astype(np.float32)


def _install_ntff_hook():
    """Shim antenv.axon_hooks with the trn_boot ctypes NTFF hook."""
    import types

    try:
        from antenv.axon_hooks import get_axon_ntff_profile_hook  # noqa
        return
    except ImportError:
        pass
    sys.path.insert(0, "/root/.axon_site/trn_agent_boot")
    import trn_boot

    hook = trn_boot._ntff_profile_via_ctypes("/opt/axon/libaxon_pjrt.so")
    mod_pkg = sys.modules.get("antenv")
    if mod_pkg is None:
        mod_pkg = types.ModuleType("antenv")
        sys.modules["antenv"] = mod_pkg
    mod = types.ModuleType("antenv.axon_hooks")
    mod.get_axon_ntff_profile_hook = lambda: hook
    mod.set_axon_ntff_profile_hook = lambda h: None
    sys.modules["antenv.axon_hooks"] = mod
    mod_pkg.axon_hooks = mod


def run_timed(inputs):
    """Run once with NTFF tracing; return exec_time_ns (or None)."""
    _install_ntff_hook()
    if "nc" not in _CACHE:
        _CACHE["nc"] = build_graph()
    nc = _CACHE["nc"]
    params = _prep_params(inputs)
    x = np.asarray(inputs["x"], np.float32).astype(ml_dtypes.bfloat16)
    in_maps = []
    for core in range(N_CORES):
        m = dict(params)
        m["xs"] = np.ascontiguousarray(x[core * IMGS : (core + 1) * IMGS])
        in_maps.append(m)
    res = run_bass_kernel_spmd(
        nc, in_maps, core_ids=list(range(N_CORES)), trace=True
    )
    print("trace profile_json:", res.profile_json)
    _CACHE["last_res"] = res
    return res.exec_time_ns


if __name__ == "__main__":
    rng = np.random.default_rng(0)
    fake = {
        "x": rng.standard_normal((16, 64, 64, 64), np.float32),
        "conv1_w": rng.standard_normal((64, 64, 3, 3), np.float32) * 0.05,
        "conv1_b": rng.standard_normal((64,), np.float32) * 0.05,
        "bn1_gamma": rng.uniform(0.5, 1.5, (64,)).astype(np.float32),
        "bn1_beta": rng.standard_normal((64,), np.float32) * 0.05,
        "bn1_mean": rng.standard_normal((64,), np.float32) * 0.05,
        "bn1_var": rng.uniform(0.5, 1.5, (64,)).astype(np.float32),
        "conv2_w": rng.standard_normal((96, 64, 3, 3), np.float32) * 0.05,
        "conv2_b": rng.standard_normal((96,), np.float32) * 0.05,
        "bn2_gamma": rng.uniform(0.5, 1.5, (96,)).astype(np.float32),
        "bn2_beta": rng.standard_normal((96,), np.float32) * 0.05,
        "bn2_mean": rng.standard_normal((96,), np.float32) * 0.05,
        "bn2_var": rng.uniform(0.5, 1.5, (96,)).astype(np.float32),
        "bases": rng.standard_normal((6, 9), np.float32),
        "coef": rng.standard_normal((128, 1024), np.float32) * 0.02,
    }
    o = kernel(**fake)
    print("out", o.shape, o.dtype)



# revision 14
# speedup vs baseline: 1.0441x; 1.0441x over previous
"""Trainium2 Bass kernel for nn_ADConv (adaptive-basis conv).

Math (per image, per pixel q=(h,w)):
  h1  = tanh(bn1(conv3x3(x)))                      # [64, H, W]
  bc  = tanh(bn2(conv3x3(h1)))                     # [96, H, W], channel = 6f+t
  PB[c,t,q]   = sum_k x[c, q+dk] * B[t,k]          # depthwise basis conv
  u[c,f,q]    = sum_t PB[c,t,q] * bc[6f+t, wq, hq] # per-pixel bilinear (DVE)
  out[o,w,h]  = sum_{c,f} coef[o, 16c+f] * u[c,f,q]

Sharding: data-parallel, batch 16 -> 2 images per NeuronCore, params
replicated. Everything computed in bf16 (fp32 PSUM accumulation).

Structure (v2):
  - conv1/conv2 for both images first (dense PE stream, warms p-state)
  - bc stored to DRAM in broadcast-friendly layout [uc, parity, f, t3, q]
    so each bcast DMA is 64 descriptors x 12KB contiguous
  - per (uc, img) block: PB matmuls -> pbt (ACT copies), then 16-f loop:
    one DVE tensor_mul per f ([128, 3072]) + 6 PE matmuls into fps PSUM
  - software pipelining: PB of block n+1 emitted before final of block n
"""

import os
import sys

import numpy as np

sys.path.insert(0, "/opt/trn_rl_repo")

import ml_dtypes

import concourse.bacc as bacc
import concourse.bass as bass
import concourse.mybir as mybir
import concourse.tile as tile
from concourse.ap import AP
from concourse.bass_utils import run_bass_kernel_spmd

BF16 = mybir.dt.bfloat16
F32 = mybir.dt.float32
AF = mybir.ActivationFunctionType
ALU = mybir.AluOpType

N_CORES = 8
IMGS = 2           # images per core
C = 64             # input channels
INTER = 64         # conv1 out channels
BCH = 96           # conv2 out channels = 16f * 6t
NT = 6             # TOTAL_BASES
NF = 16            # NUM_FA
O = 128            # output channels
H = W = 64
HP = 66            # padded spatial
Q = H * W          # 4096 pixels
RC = 8             # rows per chunk
NCHUNK = H // RC   # 8 chunks of 512 px
CH = RC * W        # 512 px per chunk
UCH = 2 * CH       # 1024-px u-chunks
NUC = NCHUNK // 2  # 4 u-chunks
BN_EPS = 1e-5

RING = 12          # bcr ring slots (f-units in flight)

_CACHE = {}


def build_graph():
    nc = bacc.Bacc(None, target_bir_lowering=False)

    xs = nc.declare_dram_parameter("xs", [IMGS, C, H, W], BF16, isOutput=False)
    w1p = nc.declare_dram_parameter("w1p", [128, 3, INTER], BF16, isOutput=False)
    w1s = nc.declare_dram_parameter("w1s", [C, 3, INTER], BF16, isOutput=False)
    s1 = nc.declare_dram_parameter("s1", [INTER, 1], F32, isOutput=False)
    b1 = nc.declare_dram_parameter("b1", [INTER, 1], F32, isOutput=False)
    w2p = nc.declare_dram_parameter("w2p", [128, 3, BCH], BF16, isOutput=False)
    w2s = nc.declare_dram_parameter("w2s", [INTER, 3, BCH], BF16, isOutput=False)
    s2 = nc.declare_dram_parameter("s2", [BCH, 1], F32, isOutput=False)
    b2 = nc.declare_dram_parameter("b2", [BCH, 1], F32, isOutput=False)
    wpbp = nc.declare_dram_parameter("wpbp", [128, 3, NT * C], BF16, isOutput=False)
    wpbs = nc.declare_dram_parameter("wpbs", [C, 3, NT * C], BF16, isOutput=False)
    coefp = nc.declare_dram_parameter("coefp", [128, NF, O], BF16, isOutput=False)
    out = nc.declare_dram_parameter("out", [IMGS, O, W, H], F32, isOutput=True)
    dbg = os.environ.get("KDBG") == "1"
    if dbg:
        dbg_pbt = nc.declare_dram_parameter(
            "dbg_pbt", [8, 128, 3, UCH], BF16, isOutput=True
        )
        dbg_th = nc.declare_dram_parameter(
            "dbg_th", [8, NF, 128, 3 * UCH], BF16, isOutput=True
        )

    with tile.TileContext(nc) as tc:
        with (
            tc.tile_pool(name="persist", bufs=1) as pp,
            tc.tile_pool(name="ring", bufs=1) as rp,
            tc.tile_pool(name="theta", bufs=4) as thp,
            tc.tile_pool(name="pbt", bufs=4) as pbp,
            tc.tile_pool(name="osb", bufs=4) as op_,
            tc.tile_pool(name="dramb", bufs=1, space=bass.MemorySpace.DRAM) as dp,
            tc.tile_pool(name="ps_conv", bufs=2, space=bass.MemorySpace.PSUM) as pcv,
            tc.tile_pool(name="ps_pb", bufs=2, space=bass.MemorySpace.PSUM) as ppb,
            tc.tile_pool(name="ps_fin", bufs=2, space=bass.MemorySpace.PSUM) as pfn,
        ):
            # ---- persistent SBUF ----
            w1psb = pp.tile([128, 3, INTER], BF16, tag="w1psb")
            w1ssb = pp.tile([C, 3, INTER], BF16, tag="w1ssb")
            w2psb = pp.tile([128, 3, BCH], BF16, tag="w2psb")
            w2ssb = pp.tile([INTER, 3, BCH], BF16, tag="w2ssb")
            wpbpsb = pp.tile([128, 3, NT * C], BF16, tag="wpbpsb")
            wpbssb = pp.tile([C, 3, NT * C], BF16, tag="wpbssb")
            coefsb = pp.tile([128, NF, O], BF16, tag="coefsb")
            s1sb = pp.tile([INTER, 1], F32, tag="s1sb")
            b1sb = pp.tile([INTER, 1], F32, tag="b1sb")
            s2sb = pp.tile([BCH, 1], F32, tag="s2sb")
            b2sb = pp.tile([BCH, 1], F32, tag="b2sb")
            nc.sync.dma_start(w1psb[:], w1p[:])
            nc.sync.dma_start(w1ssb[:], w1s[:])
            nc.sync.dma_start(w2psb[:], w2p[:])
            nc.sync.dma_start(w2ssb[:], w2s[:])
            nc.gpsimd.dma_start(wpbpsb[:], wpbp[:])
            nc.gpsimd.dma_start(wpbssb[:], wpbs[:])
            nc.gpsimd.dma_start(coefsb[:], coefp[:])
            nc.scalar.dma_start(s1sb[:], s1[:])
            nc.scalar.dma_start(b1sb[:], b1[:])
            nc.scalar.dma_start(s2sb[:], s2[:])
            nc.scalar.dma_start(b2sb[:], b2[:])

            xpad = []
            hpad = []
            bcs = []
            for i in range(IMGS):
                xp = pp.tile([128, HP, HP], BF16, tag=f"xpad{i}", name=f"xpad{i}")
                hp = pp.tile([128, HP, HP], BF16, tag=f"hpad{i}", name=f"hpad{i}")
                bi = pp.tile([BCH, H, W], BF16, tag=f"bc{i}", name=f"bc{i}")
                nc.gpsimd.memset(xp[:], 0.0)
                nc.gpsimd.memset(hp[:], 0.0)
                nc.sync.dma_start(xp[0:64, 1 : H + 1, 1 : W + 1], xs[i])
                nc.scalar.dma_start(xp[64:128, 1 : H + 1, 0:W], xs[i])
                xpad.append(xp)
                hpad.append(hp)
                bcs.append(bi)

            # bcr ring: one big tile, slot = [128, 3072] (parity split on
            # partition halves, free = [t3, 1024])
            ring = rp.tile([128, RING, 3 * UCH], BF16, tag="ring", name="ring")
            # bc in DRAM, broadcast-friendly: [uc, parity, f, t3, q1024]
            bcd = [
                dp.tile([NUC, 2, NF, 3, UCH], BF16, tag=f"bcd{i}", name=f"bcd{i}")
                for i in range(IMGS)
            ]

            # ---- conv phase: conv1 both images, then conv2 both images ----
            for i in range(IMGS):
                for g in range(NCHUNK):
                    cpsf = pcv.tile([BCH, RC, W], F32, tag="convps")
                    cps = cpsf[0:INTER]
                    h0 = g * RC
                    for m in range(6):
                        ki = m % 3
                        pair = m < 3
                        lhsT = w1psb[:, ki, :] if pair else w1ssb[:, ki, :]
                        if pair:
                            rhs = xpad[i][:, h0 + ki : h0 + ki + RC, 0:W]
                        else:
                            rhs = xpad[i][0:64, h0 + ki : h0 + ki + RC, 2 : 2 + W]
                        nc.tensor.matmul(
                            cps, lhsT, rhs, start=(m == 0), stop=(m == 5)
                        )
                    nc.scalar.activation(
                        hpad[i][0:64, h0 + 1 : h0 + 1 + RC, 1 : W + 1],
                        cps,
                        AF.Tanh,
                        bias=b1sb[:],
                        scale=s1sb[:],
                    )
                    nc.scalar.activation(
                        hpad[i][64:128, h0 + 1 : h0 + 1 + RC, 0:W],
                        cps,
                        AF.Tanh,
                        bias=b1sb[:],
                        scale=s1sb[:],
                    )
            for i in range(IMGS):
                for g in range(NCHUNK):
                    cps = pcv.tile([BCH, RC, W], F32, tag="convps")
                    h0 = g * RC
                    for m in range(6):
                        ki = m % 3
                        pair = m < 3
                        lhsT = w2psb[:, ki, :] if pair else w2ssb[:, ki, :]
                        if pair:
                            rhs = hpad[i][:, h0 + ki : h0 + ki + RC, 0:W]
                        else:
                            rhs = hpad[i][0:64, h0 + ki : h0 + ki + RC, 2 : 2 + W]
                        nc.tensor.matmul(
                            cps[:], lhsT, rhs, start=(m == 0), stop=(m == 5)
                        )
                    # transposed store: bcs[ch, a, b] = conv2out[ch, b, a]
                    # so bcs free-order == PB pixel order q=(h, w)
                    nc.scalar.activation(
                        bcs[i][:, :, h0 : h0 + RC].transpose([0, 2, 1]),
                        cps[:],
                        AF.Tanh,
                        bias=b2sb[:],
                        scale=s2sb[:],
                    )
                # store bc to DRAM in bcast layout; 6 dma_starts (per t).
                # bcs partition order is permuted to [parity, t3, f] (see
                # _prep_params) so each slice is 16 contiguous partitions.
                for t in range(NT):
                    parity = t % 2
                    t3 = t // 2
                    row0 = parity * 48 + t3 * NF
                    sview = bcs[i][row0 : row0 + NF, :, :]
                    # dst iterated (f, uc, q) to match src (partition, free)
                    dview = bcd[i][:, parity, :, t3, :].transpose([1, 0, 2])
                    iss = (nc.sync, nc.gpsimd, nc.scalar)[t % 3]
                    iss.dma_start(dview, sview)

            # ---- final phase: per (uc, img) blocks, software-pipelined ----
            blocks = [(uc, i) for uc in range(NUC) for i in range(IMGS)]
            nb = len(blocks)
            pbt_tiles = {}
            fcount = 0

            def emit_quad(j):
                # broadcast DMA for f-units [4j, 4j+4): 2 dma_starts (one
                # per parity), each 64 descriptors x 12KB contiguous.
                # NOTE: emission order defines ring-slot semantics — a quad
                # must be emitted only after the TTs of the previous users
                # of its slots.
                if not (0 <= j < (len(blocks) * NF) // 4):
                    return
                u0 = 4 * j
                bidx, f0 = divmod(u0, NF)
                uc, i = blocks[bidx]
                s0 = u0 % RING
                for parity in range(2):
                    srcap = bcd[i][uc, parity, f0 : f0 + 4, :, :].partition_broadcast(64)
                    dstap = ring[parity * 64 : (parity + 1) * 64, s0 : s0 + 4, :]
                    iss = (nc.sync, nc.gpsimd, nc.scalar)[(j * 2 + parity) % 3]
                    iss.dma_start(dstap, srcap)

            def emit_pb(bidx):
                # PB matmuls + ACT copies -> pbt tile for block bidx
                uc, i = blocks[bidx]
                pbt = pbp.tile([128, 3, UCH], BF16, tag="pbt", name=f"pbt{bidx}")
                pbt_tiles[bidx] = pbt
                for j3 in range(3):
                    for half in range(2):
                        h0 = (uc * 2 + half) * RC
                        pps = ppb.tile([128, RC, W], F32, tag="pbps", name="pps")
                        for m in range(6):
                            ki = m % 3
                            pair = m < 3
                            if pair:
                                lhsT = wpbpsb[:, ki, j3 * 128 : (j3 + 1) * 128]
                                rhs = xpad[i][:, h0 + ki : h0 + ki + RC, 0:W]
                            else:
                                lhsT = wpbssb[:, ki, j3 * 128 : (j3 + 1) * 128]
                                rhs = xpad[i][0:64, h0 + ki : h0 + ki + RC, 2 : 2 + W]
                            nc.tensor.matmul(
                                pps[:], lhsT, rhs, start=(m == 0), stop=(m == 5)
                            )
                        nc.scalar.copy(
                            pbt[:, j3, half * CH : (half + 1) * CH],
                            pps[:].opt(),
                        )

            def emit_final(bidx):
                nonlocal fcount
                uc, i = blocks[bidx]
                pbt = pbt_tiles.pop(bidx)
                fps = pfn.tile([O, 2, CH], F32, tag="finps", name=f"fps{bidx}")
                for f in range(NF):
                    u = bidx * NF + f
                    if f % 4 == 0:
                        emit_quad((u + 8) // 4)
                    slot = u % RING
                    theta = thp.tile([128, 3 * UCH], BF16, tag="theta", name="theta")
                    rview = ring[:, slot, :]
                    nc.vector.tensor_mul(theta[:], pbt[:].opt(), rview)
                    if dbg:
                        if f == 0:
                            nc.gpsimd.dma_start(dbg_pbt[bidx], pbt[:])
                        nc.gpsimd.dma_start(dbg_th[bidx, f], theta[:])
                    for j3 in range(3):
                        for half in range(2):
                            nc.tensor.matmul(
                                fps[:, half],
                                coefsb[:, f, :],
                                theta[:, j3 * UCH + half * CH : j3 * UCH + (half + 1) * CH],
                                start=(f == 0 and j3 == 0),
                                stop=(f == NF - 1 and j3 == 2),
                            )
                    fcount += 1
                for half in range(2):
                    ch = uc * 2 + half
                    h0 = ch * RC
                    osb = op_.tile([O, W, RC], F32, tag="osb", name="osb")
                    nc.scalar.copy(
                        osb[:].transpose([0, 2, 1]),
                        fps[:, half].opt(),
                    )
                    iss = (nc.sync, nc.gpsimd)[half]
                    iss.dma_start(out[i, :, :, h0 : h0 + RC], osb[:])

            # pipelined emission: prime 2 quads, PB one block ahead of final
            emit_quad(0)
            emit_quad(1)
            emit_pb(0)
            for b in range(nb):
                if b + 1 < nb:
                    emit_pb(b + 1)
                emit_final(b)

    nc.compile()
    return nc


def _prep_params(inputs):
    bf16 = ml_dtypes.bfloat16
    f32 = np.float32
    c1w = np.asarray(inputs["conv1_w"], f32)
    c2w = np.asarray(inputs["conv2_w"], f32)
    bases = np.asarray(inputs["bases"], f32)
    coef = np.asarray(inputs["coef"], f32)

    s1 = np.asarray(inputs["bn1_gamma"], f32) / np.sqrt(
        np.asarray(inputs["bn1_var"], f32) + BN_EPS
    )
    b1 = (np.asarray(inputs["conv1_b"], f32) - np.asarray(inputs["bn1_mean"], f32)) * s1 + np.asarray(
        inputs["bn1_beta"], f32
    )
    s2 = np.asarray(inputs["bn2_gamma"], f32) / np.sqrt(
        np.asarray(inputs["bn2_var"], f32) + BN_EPS
    )
    b2 = (np.asarray(inputs["conv2_b"], f32) - np.asarray(inputs["bn2_mean"], f32)) * s2 + np.asarray(
        inputs["bn2_beta"], f32
    )

    # conv2 output-channel permutation: bcs row r' = parity*48 + t3*16 + f
    # holds original channel 6f + 2*t3 + parity
    perm = np.empty(BCH, np.int64)
    for rp in range(BCH):
        f_ = rp % NF
        t3_ = (rp // NF) % 3
        par_ = rp // 48
        perm[rp] = NT * f_ + 2 * t3_ + par_
    c2wp = c2w[perm]
    s2 = s2[perm]
    b2 = b2[perm]

    w1pk = np.zeros((128, 3, INTER), f32)
    w1sk = np.zeros((C, 3, INTER), f32)
    w2pk = np.zeros((128, 3, BCH), f32)
    w2sk = np.zeros((INTER, 3, BCH), f32)
    for ki in range(3):
        w1pk[0:64, ki] = c1w[:, :, ki, 0].T
        w1pk[64:128, ki] = c1w[:, :, ki, 1].T
        w1sk[:, ki] = c1w[:, :, ki, 2].T
        w2pk[0:64, ki] = c2wp[:, :, ki, 0].T
        w2pk[64:128, ki] = c2wp[:, :, ki, 1].T
        w2sk[:, ki] = c2wp[:, :, ki, 2].T

    wpbpk = np.zeros((128, 3, NT * C), f32)
    wpbsk = np.zeros((C, 3, NT * C), f32)
    for t in range(NT):
        for c in range(C):
            for ki in range(3):
                wpbpk[c, ki, t * C + c] = bases[t, 3 * ki + 0]
                wpbpk[64 + c, ki, t * C + c] = bases[t, 3 * ki + 1]
                wpbsk[c, ki, t * C + c] = bases[t, 3 * ki + 2]

    cview = coef.reshape(O, C, NF)  # coef[o, 16c+f]
    coefp64 = np.ascontiguousarray(cview.transpose(1, 2, 0))  # [c, f, o]
    coefp = np.concatenate([coefp64, coefp64], axis=0)  # [128, f, o]

    return {
        "w1p": w1pk.astype(bf16),
        "w1s": w1sk.astype(bf16),
        "s1": s1.reshape(-1, 1).astype(f32),
        "b1": b1.reshape(-1, 1).astype(f32),
        "w2p": w2pk.astype(bf16),
        "w2s": w2sk.astype(bf16),
        "s2": s2.reshape(-1, 1).astype(f32),
        "b2": b2.reshape(-1, 1).astype(f32),
        "wpbp": wpbpk.astype(bf16),
        "wpbs": wpbsk.astype(bf16),
        "coefp": coefp.astype(bf16),
    }


def kernel(**inputs):
    if "nc" not in _CACHE:
        _CACHE["nc"] = build_graph()
    nc = _CACHE["nc"]

    params = _prep_params(inputs)
    x = np.asarray(inputs["x"], np.float32).astype(ml_dtypes.bfloat16)

    in_maps = []
    for core in range(N_CORES):
        m = dict(params)
        m["xs"] = np.ascontiguousarray(x[core * IMGS : (core + 1) * IMGS])
        in_maps.append(m)

    res = run_bass_kernel_spmd(nc, in_maps, core_ids=list(range(N_CORES)))
    outs = [r["out"] for r in res.results]
    return np.concatenate(outs, axis=0).astype(np.float32)


def _install_ntff_hook():
    """Shim antenv.axon_hooks with the trn_boot ctypes NTFF hook."""
    import types

    try:
        from antenv.axon_hooks import get_axon_ntff_profile_hook  # noqa
        return
    except ImportError:
        pass
    sys.path.insert(0, "/root/.axon_site/trn_agent_boot")
    import trn_boot

    hook = trn_boot._ntff_profile_via_ctypes("/opt/axon/libaxon_pjrt.so")
    mod_pkg = sys.modules.get("antenv")
    if mod_pkg is None:
        mod_pkg = types.ModuleType("antenv")
        sys.modules["antenv"] = mod_pkg
    mod = types.ModuleType("antenv.axon_hooks")
    mod.get_axon_ntff_profile_hook = lambda: hook
    mod.set_axon_ntff_profile_hook = lambda h: None
    sys.modules["antenv.axon_hooks"] = mod
    mod_pkg.axon_hooks = mod


def run_timed(inputs):
    """Run once with NTFF tracing; return exec_time_ns (or None)."""
    _install_ntff_hook()
    if "nc" not in _CACHE:
        _CACHE["nc"] = build_graph()
    nc = _CACHE["nc"]
    params = _prep_params(inputs)
    x = np.asarray(inputs["x"], np.float32).astype(ml_dtypes.bfloat16)
    in_maps = []
    for core in range(N_CORES):
        m = dict(params)
        m["xs"] = np.ascontiguousarray(x[core * IMGS : (core + 1) * IMGS])
        in_maps.append(m)
    res = run_bass_kernel_spmd(
        nc, in_maps, core_ids=list(range(N_CORES)), trace=True
    )
    print("trace profile_json:", res.profile_json)
    _CACHE["last_res"] = res
    return res.exec_time_ns


if __name__ == "__main__":
    rng = np.random.default_rng(0)
    fake = {
        "x": rng.standard_normal((16, 64, 64, 64)).astype(np.float32),
        "conv1_w": (rng.standard_normal((64, 64, 3, 3)) * 0.05).astype(np.float32),
        "conv1_b": (rng.standard_normal((64,)) * 0.05).astype(np.float32),
        "bn1_gamma": rng.uniform(0.5, 1.5, (64,)).astype(np.float32),
        "bn1_beta": (rng.standard_normal((64,)) * 0.05).astype(np.float32),
        "bn1_mean": (rng.standard_normal((64,)) * 0.05).astype(np.float32),
        "bn1_var": rng.uniform(0.5, 1.5, (64,)).astype(np.float32),
        "conv2_w": (rng.standard_normal((96, 64, 3, 3)) * 0.05).astype(np.float32),
        "conv2_b": (rng.standard_normal((96,)) * 0.05).astype(np.float32),
        "bn2_gamma": rng.uniform(0.5, 1.5, (96,)).astype(np.float32),
        "bn2_beta": (rng.standard_normal((96,)) * 0.05).astype(np.float32),
        "bn2_mean": (rng.standard_normal((96,)) * 0.05).astype(np.float32),
        "bn2_var": rng.uniform(0.5, 1.5, (96,)).astype(np.float32),
        "bases": rng.standard_normal((6, 9)).astype(np.float32),
        "coef": (rng.standard_normal((128, 1024)) * 0.02).astype(np.float32),
    }
    o = kernel(**fake)
    print("out", o.shape, o.dtype)
